# revision 27
# baseline (speedup 1.0000x reference)
"""DSCLRCN Trainium2 kernel: 4x BiLSTM grid scans + 1x1 conv + bilinear resize + softmax.

Self-contained: shards batch N=8 across 8 NeuronCores (1 sample/core),
builds one SPMD Bass program, runs via run_bass_kernel_spmd, gathers output.
"""
import os
import sys

sys.path.insert(0, '/opt/trn_rl_repo')

import numpy as np
import ml_dtypes

import concourse.bass as bass
import concourse.tile as tile
from concourse import bacc, mybir
from concourse.bass_utils import run_bass_kernel_spmd

HID = 128
GH, GW = 60, 80
IN_H, IN_W = 480, 640
NCORES = 8

F32 = mybir.dt.float32
BF16 = mybir.dt.bfloat16
AF = mybir.ActivationFunctionType
ALU = mybir.AluOpType
bfnp = ml_dtypes.bfloat16

# pass configs: (name, din, T, B) ; T = scan length, B = batch rows per step
# h-pass scans along x (T=80, B=60); v-pass scans along y (T=60, B=80)
PASSES = [
    ("h1", 512, GW, GH),
    ("v1", 256, GH, GW),
    ("h2", 256, GW, GH),
    ("v2", 256, GH, GW),
]

# gate slot order used on-chip: (f, i, o, g); pytorch row order is (i, f, g, o)
# rows of wih/whh: i=0:128 f=128:256 g=256:384 o=384:512 -> perm picks (f,i,o,g)
GATE_PERM = np.concatenate([
    np.arange(128, 256),   # f
    np.arange(0, 128),     # i
    np.arange(384, 512),   # o
    np.arange(256, 384),   # g
])

_cache = {}


def _resize_mats():
    def coords(insz, outsz):
        pos = np.linspace(0.0, insz - 1.0, outsz, dtype=np.float32)
        i0 = np.clip(np.floor(pos).astype(np.int32), 0, insz - 2)
        return i0, (pos - i0).astype(np.float32)

    x0, fx = coords(GW, IN_W)
    y0, fy = coords(GH, IN_H)
    rx = np.zeros((GW, IN_W), np.float32)      # rx[x, i]
    rx[x0, np.arange(IN_W)] += 1.0 - fx
    rx[x0 + 1, np.arange(IN_W)] += fx
    ryT = np.zeros((GH, IN_H), np.float32)     # ryT[y, j]
    ryT[y0, np.arange(IN_H)] += 1.0 - fy
    ryT[y0 + 1, np.arange(IN_H)] += fy
    return rx, ryT


def _prep_shared(inputs):
    """Host-side layout prep of weights (shared by all cores)."""
    sh = {}
    for nm, din, T, B in PASSES:
        wih = inputs[nm + "_wih"]   # [2, 512, din]
        whh = inputs[nm + "_whh"]   # [2, 512, 128]
        bih = inputs[nm + "_bih"]   # [2, 512]
        bhh = inputs[nm + "_bhh"]
        wihT = np.ascontiguousarray(np.transpose(wih[:, GATE_PERM, :], (0, 2, 1)))  # [2, din, 512]
        whhT = np.ascontiguousarray(np.transpose(whh[:, GATE_PERM, :], (0, 2, 1)))  # [2, 128, 512]
        b = (bih + bhh)[:, GATE_PERM]                                               # [2, 512]
        if nm == "h1":
            sh["wihT_h1"] = wihT.astype(np.float32)
        else:
            sh["wihT_" + nm] = wihT.astype(bfnp)
        sh["whhT_" + nm] = whhT.astype(bfnp)
        # per-gate-chunk bias columns for proj evacuation: [2, 128, 4]
        bc = b.reshape(2, 4, 128).transpose(0, 2, 1)
        sh["bcol_" + nm] = np.ascontiguousarray(bc).astype(np.float32)
    sh["fc1_wT"] = np.ascontiguousarray(inputs["fc1_w"].T).astype(np.float32)       # [128, 512]
    sh["fc1_bcol"] = np.ascontiguousarray(inputs["fc1_b"].reshape(4, 128).T).astype(np.float32)
    sh["fcr_wT"] = np.ascontiguousarray(inputs["fcr_w"].T).astype(np.float32)       # [128, 256]
    sh["fcr_bcol"] = np.ascontiguousarray(inputs["fcr_b"].reshape(2, 128).T).astype(np.float32)
    sh["conv_wT"] = np.ascontiguousarray(
        inputs["conv_w"].reshape(2, 128, 1)).astype(bfnp)                           # [2,128,1]
    sh["conv_b"] = inputs["conv_b"].reshape(1, 1).astype(np.float32)
    rx, ryT = _resize_mats()
    sh["rx"] = rx
    sh["ryT"] = ryT
    sh["ident"] = np.eye(128, dtype=np.float32).astype(bfnp)
    sh["vstamp"] = np.zeros((1, _source_stamp()), np.float32)
    return sh


def _declare_inputs(nc):
    d = {}

    def di(name, shape, dt):
        d[name] = nc.dram_tensor(name, list(shape), dt, kind="ExternalInput").ap()

    di("lf", (512, GW, GH), F32)          # x-major per-sample features
    di("ctx", (128, 1), F32)
    di("wihT_h1", (2, 512, 512), F32)
    for nm in ("v1", "h2", "v2"):
        di("wihT_" + nm, (2, 256, 512), BF16)
    for nm, _, _, _ in PASSES:
        di("whhT_" + nm, (2, 128, 512), BF16)
        di("bcol_" + nm, (2, 128, 4), F32)
    di("fc1_wT", (128, 512), F32)
    di("fc1_bcol", (128, 4), F32)
    di("fcr_wT", (128, 256), F32)
    di("fcr_bcol", (128, 2), F32)
    di("conv_wT", (2, 128, 1), BF16)
    di("conv_b", (1, 1), F32)
    di("rx", (GW, IN_W), F32)
    di("ryT", (GH, IN_H), F32)
    di("ident", (128, 128), BF16)
    # version stamp: the neuron compile cache keys on HLO shapes only (the BIR
    # rides through a side-channel hook), so encode the kernel source hash in a
    # dummy input's shape to avoid stale-NEFF cache hits across kernel versions.
    di("vstamp", (1, _source_stamp()), F32)
    return d


def _source_stamp():
    import zlib
    with open(os.path.abspath(__file__), "rb") as f:
        return (zlib.crc32(f.read()) % 509) + 2


def _ctx_pass_setup(nc, cxp, ps, din, wihT_tiles, bcol_t, cvec):
    """Per (pass, dir): gx_ctx col [128,4] -> mini LSTM step -> h0/c0 cols [128,1] f32.

    wihT_tiles: list of [128,512] lhsT tiles; cvec: [128, KT] (dtype matches wihT).
    bcol_t: [128, 4] f32 bias columns (bih+bhh per gate slot).
    """
    KT = din // 128
    gps = ps.tile([128, 2, 512], F32, tag="pst")
    gxp = gps[:, 0, 0:4]
    for g in range(4):
        for kt in range(KT):
            nc.tensor.matmul(gxp[:, g:g + 1], wihT_tiles[kt][:, g * 128:(g + 1) * 128],
                             cvec[:, kt:kt + 1], start=(g == 0 and kt == 0),
                             stop=(g == 3 and kt == KT - 1), skip_group_check=True)
    gxc = cxp.tile([128, 4], F32, tag="cxgxc")
    nc.vector.tensor_tensor(gxc[:], gxp, bcol_t[:], ALU.add)
    sig = cxp.tile([128, 3], F32, tag="cxsig")
    tg = cxp.tile([128, 1], F32, tag="cxtg")
    nc.scalar.activation(sig[:], gxc[:, 0:3], AF.Sigmoid)
    nc.scalar.activation(tg[:], gxc[:, 3:4], AF.Tanh)
    c0 = cxp.tile([128, 1], F32, tag="cxc0", bufs=2)
    nc.vector.tensor_tensor(c0[:], sig[:, 1:2], tg[:], ALU.mult)
    tc0 = cxp.tile([128, 1], F32, tag="cxtc0")
    nc.scalar.activation(tc0[:], c0[:], AF.Tanh)
    h0 = cxp.tile([128, 1], F32, tag="cxh0", bufs=2)
    nc.vector.tensor_tensor(h0[:], sig[:, 2:3], tc0[:], ALU.mult)
    return h0, c0


def _bcast_state(nc, pp, B, hcols, ccols, tag):
    """Build h_init/c_init [128, 2B] bf16 broadcast from per-dir columns."""
    h_init = pp.tile([128, 2 * B], BF16, tag=tag + "h")
    c_init = pp.tile([128, 2 * B], BF16, tag=tag + "c")
    for t in (h_init, c_init):
        nc.vector.memset(t[:], 0.0)
    for d in range(2):
        sl = slice(d * B, (d + 1) * B)
        nc.vector.tensor_scalar(h_init[:, sl], h_init[:, sl], hcols[d][:], None, ALU.add)
        nc.vector.tensor_scalar(c_init[:, sl], c_init[:, sl], ccols[d][:], None, ALU.add)
    return h_init, c_init


def build_program(debug=False):
    nc = bacc.Bacc(trn_type="TRN2", name="dsclrcn")
    dins = _declare_inputs(nc)
    out_dram = nc.dram_tensor("out", [IN_H, IN_W], F32, kind="ExternalOutput").ap()

    from contextlib import ExitStack
    with ExitStack() as st:
        tc = st.enter_context(tile.TileContext(nc))
        pp = st.enter_context(tc.tile_pool(name="persist", bufs=1))
        ps = st.enter_context(tc.tile_pool(name="ps", bufs=4, space="PSUM"))
        sp = st.enter_context(tc.tile_pool(name="scan", bufs=3))

        # ---------- persistent sbuf loads ----------
        def load(name, shape, dt, src):
            t = pp.tile(shape, dt, tag=name)
            nc.sync.dma_start(t[:], src)
            return t

        ident = load("ident", [128, 128], BF16, dins["ident"][:])
        vstamp = load("vstamp", [1, _source_stamp()], F32, dins["vstamp"][:])
        wihs = {}
        for nm in ("v1", "h2", "v2"):
            wihs[nm] = [load(f"wih_{nm}_{k}", [128, 512], BF16,
                             dins["wihT_" + nm][0 if k < 2 else 1, k % 2 * 128:(k % 2 + 1) * 128, :])
                        for k in range(4)]  # dirs x 2 ktiles
        whhs = {nm: [load(f"whh_{nm}_{d}", [128, 512], BF16, dins["whhT_" + nm][d])
                     for d in range(2)] for nm, _, _, _ in PASSES}
        bcols = {nm: [load(f"bcol_{nm}_{d}", [128, 4], F32, dins["bcol_" + nm][d])
                      for d in range(2)] for nm, _, _, _ in PASSES}
        fc1_wT = load("fc1_wT", [128, 512], F32, dins["fc1_wT"][:])
        fc1_bcol = load("fc1_bcol", [128, 4], F32, dins["fc1_bcol"][:])
        fcr_wT = load("fcr_wT", [128, 256], F32, dins["fcr_wT"][:])
        fcr_bcol = load("fcr_bcol", [128, 2], F32, dins["fcr_bcol"][:])
        ctx_col = load("ctx", [128, 1], F32, dins["ctx"][:])

        # ---------- big persistent buffers ----------
        gx_flat = pp.tile([128, 38400], BF16, tag="gx")        # [p, T*8B] viewed per pass
        hbufA = pp.tile([128, 2, GH * GW], BF16, tag="hbufA")
        hbufB = pp.tile([128, 2, GH * GW], BF16, tag="hbufB")

        dbg_tiles = {}

        # ================= proj + scan per pass =================
        def proj_pass(nm, din, T, B, rhs_tiles_fn, wt_list, KT, evac_idx):
            """Input projection: fills gxv[:, 0:T, 0:8B].

            rhs_tiles_fn(ch) -> list of KT rhs APs [128, tc*B] for chunk ch.
            wt_list[d][kt] -> lhsT [128,512] tile.
            """
            gxv = gx_flat[:, 0:T * 8 * B].rearrange("p (t s) -> p t s", t=T)
            tch = 480 // B          # t's per chunk (6 or 8)
            nch = T // tch          # chunks (10)
            N = tch * B             # 480
            for ch in range(nch):
                rhs = rhs_tiles_fn(ch)
                for d in range(2):
                    for g in range(4):
                        P = ps.tile([128, 2, 512], F32, tag="pst")
                        Pv = P.rearrange("p a b -> p (a b)")[:, 0:N]
                        lhs = wt_list[d][0][:, g * 128:(g + 1) * 128]
                        nc.tensor.matmul(Pv, lhs, rhs[0], start=True, stop=(KT == 1))
                        for kt in range(1, KT):
                            lhs = wt_list[d][kt][:, g * 128:(g + 1) * 128]
                            nc.tensor.matmul(Pv, lhs, rhs[kt], start=False, stop=(kt == KT - 1))
                        dst = gxv[:, ch * tch:(ch + 1) * tch, (4 * d + g) * B:(4 * d + g + 1) * B]
                        bias = bcols[nm][d][:, g:g + 1]
                        src3 = P.rearrange("p a b -> p (a b)")[:, 0:N].rearrange("p (t b) -> p t b", t=tch)
                        if evac_idx[0] % 2 == 0:
                            nc.scalar.activation(dst, src3, AF.Identity, bias=bias)
                        else:
                            nc.vector.tensor_scalar(dst, src3, bias, None, ALU.add)
                        evac_idx[0] += 1

        def scan_pass(nm, T, B, h_out):
            whh = whhs[nm]
            h_init, c_state = inits[nm]
            gx_all = gx_flat[:, 0:T * 8 * B].rearrange("p (t s) -> p t s", t=T)
            hv = h_out.rearrange("p d (b t) -> p d b t", t=T)
            SB = 2 * B
            for t in range(T):
                tf, tb = t, T - 1 - t
                P = ps.tile([128, 2, 512], F32, tag="pst")
                # identity preloads: bank d = dir d gates [f i o g] (4B each)
                nc.tensor.matmul(P[:, 0, 0:4 * B], ident[:], gx_all[:, tf, 0:4 * B],
                                 start=True, stop=False, skip_group_check=True)
                nc.tensor.matmul(P[:, 1, 0:4 * B], ident[:], gx_all[:, tb, 4 * B:8 * B],
                                 start=True, stop=False, skip_group_check=True)
                # recurrent matmuls
                for d in range(2):
                    if t == 0:
                        rhs = h_init[:, d * B:(d + 1) * B]
                    elif d == 0:
                        rhs = hv[:, 0, :, t - 1]
                    else:
                        rhs = hv[:, 1, :, T - t]
                    for g in range(4):
                        nc.tensor.matmul(P[:, d, g * B:(g + 1) * B],
                                         whh[d][:, g * 128:(g + 1) * 128], rhs,
                                         start=False, stop=(g == 3), skip_group_check=True)
                # elementwise (cross-bank 2D APs: [p, 2, x])
                S = sp.tile([128, 2, 3 * B], BF16, tag="S")
                nc.scalar.activation(S[:], P[:, :, 0:3 * B], AF.Sigmoid)
                TG = sp.tile([128, 2, 80], BF16, tag="TG")
                nc.scalar.activation(TG[:, :, 0:B], P[:, :, 3 * B:4 * B], AF.Tanh)
                cs = c_state.rearrange("p (d b) -> p d b", d=2)
                PP = sp.tile([128, 2, 80], BF16, tag="PP")
                nc.vector.tensor_tensor(PP[:, :, 0:B], S[:, :, B:2 * B], TG[:, :, 0:B], ALU.mult)
                nc.vector.tensor_tensor(cs, cs, S[:, :, 0:B], ALU.mult)
                nc.vector.tensor_tensor(cs, cs, PP[:, :, 0:B], ALU.add)
                TC = sp.tile([128, 160], BF16, tag="TC")
                nc.scalar.activation(TC[:, 0:SB], c_state[:, 0:SB], AF.Tanh)
                nc.vector.tensor_tensor(hv[:, 0, :, tf], S[:, 0, 2 * B:3 * B], TC[:, 0:B], ALU.mult)
                nc.vector.tensor_tensor(hv[:, 1, :, tb], S[:, 1, 2 * B:3 * B], TC[:, B:SB], ALU.mult)

        # ----- ctx path + pass 1 (h1) inside closable pools -----
        inits = {}
        with tc.tile_pool(name="h1w", bufs=1) as h1wp:
            wih1 = [h1wp.tile([128, 512], F32, name=f"wih1_{k}", tag=f"wih1_{k}")
                    for k in range(8)]
            for k in range(8):
                nc.sync.dma_start(wih1[k][:],
                                  dins["wihT_h1"][k // 4, (k % 4) * 128:(k % 4 + 1) * 128, :])

            with tc.tile_pool(name="cxp", bufs=2) as cxp:
                # context vectors as columns: c1 [128,4] f32, cr [128,2] (f32+bf16)
                c1ps = ps.tile([128, 2, 512], F32, tag="pst")
                for m in range(4):
                    nc.tensor.matmul(c1ps[:, 0, m:m + 1], fc1_wT[:, m * 128:(m + 1) * 128],
                                     ctx_col[:], start=(m == 0), stop=(m == 3),
                                     skip_group_check=True)
                c1_col = pp.tile([128, 4], F32, tag="c1col")
                nc.vector.tensor_tensor(c1_col[:], c1ps[:, 0, 0:4], fc1_bcol[:], ALU.add)

                crps = ps.tile([128, 2, 512], F32, tag="pst")
                for m in range(2):
                    nc.tensor.matmul(crps[:, 0, m:m + 1], fcr_wT[:, m * 128:(m + 1) * 128],
                                     ctx_col[:], start=(m == 0), stop=(m == 1),
                                     skip_group_check=True)
                cr_colf = pp.tile([128, 2], F32, tag="crcolf")
                nc.vector.tensor_tensor(cr_colf[:], crps[:, 0, 0:2], fcr_bcol[:], ALU.add)
                cr_col = pp.tile([128, 2], BF16, tag="crcol")
                nc.vector.tensor_copy(cr_col[:], cr_colf[:])

                # per-pass ctx mini-steps -> initial state columns
                for nm, din, T, B in PASSES:
                    hcols, ccols = [], []
                    for d in range(2):
                        if nm == "h1":
                            wt = [wih1[d * 4 + k] for k in range(4)]
                            cv = c1_col
                        else:
                            wt = [wihs[nm][d * 2 + k] for k in range(2)]
                            cv = cr_col
                        h0, c0 = _ctx_pass_setup(nc, cxp, ps, din, wt, bcols[nm][d], cv)
                        hcols.append(h0)
                        ccols.append(c0)
                    inits[nm] = _bcast_state(nc, pp, B, hcols, ccols, f"init_{nm}")

            with tc.tile_pool(name="lfp", bufs=2) as lfp:
                lf = dins["lf"]

                def h1_rhs(ch):
                    t0 = ch * 8
                    lt = lfp.tile([128, 4, 8 * GH], F32, tag="lfch")
                    for kt in range(4):
                        nc.sync.dma_start(lt[:, kt, :],
                                          lf[kt * 128:(kt + 1) * 128, t0:t0 + 8, :].rearrange("p a b -> p (a b)"))
                    return [lt[:, kt, :] for kt in range(4)]

                wl = [[wih1[d * 4 + k] for k in range(4)] for d in range(2)]
                proj_pass("h1", 512, GW, GH, h1_rhs, wl, 4, [0])
        if debug:
            dgx = nc.dram_tensor("dbg_gx1", [128, 38400], BF16, kind="ExternalOutput").ap()
            nc.sync.dma_start(dgx[:], gx_flat[:])
            dini = nc.dram_tensor("dbg_init1", [128, 4 * GH], F32, kind="ExternalOutput").ap()
            hi1, ci1 = inits["h1"]
            ivt = pp.tile([128, 4 * GH], F32, tag="ivt")
            nc.vector.tensor_copy(ivt[:, 0:2 * GH], hi1[:])
            nc.vector.tensor_copy(ivt[:, 2 * GH:4 * GH], ci1[:])
            nc.sync.dma_start(dini[:], ivt[:])
        scan_pass("h1", GW, GH, hbufA)

        # ----- passes 2..4 -----
        prev = hbufA
        nxt = hbufB
        for nm, din, T, B in PASSES[1:]:
            tch = 480 // B
            src = prev.rearrange("p d (t b) -> p d t b", t=T)

            def mk_rhs(src=src, tch=tch):
                def f(ch):
                    return [src[:, kt, ch * tch:(ch + 1) * tch, :].rearrange("p a b -> p (a b)")
                            for kt in range(2)]
                return f

            wl = [[wihs[nm][d * 2 + k] for k in range(2)] for d in range(2)]
            proj_pass(nm, din, T, B, mk_rhs(), wl, 2, [0])
            scan_pass(nm, T, B, nxt)
            prev, nxt = nxt, prev

        h4 = prev  # [128, 2, 4800] layout [d, x, y] (B=80 batch-major, t=y minor)

        if debug:
            for name, buf in (("dbg_h1", hbufA), ("dbg_h4", h4)):
                dt = nc.dram_tensor(name, [128, 2, GH * GW], BF16, kind="ExternalOutput").ap()
                nc.sync.dma_start(dt[:], buf[:])

        # ================= conv 1x1 -> G [1, 4800] =================
        with tc.tile_pool(name="tail", bufs=1) as tp:
            conv_wT = tp.tile([128, 2], BF16, tag="conv_wT")
            for kt in range(2):
                nc.sync.dma_start(conv_wT[:, kt:kt + 1], dins["conv_wT"][kt])
            conv_b = load("conv_b", [1, 1], F32, dins["conv_b"][:])
            rx_t = load("rx", [GW, IN_W], F32, dins["rx"][:])
            ryT_t = load("ryT", [GH, IN_H], F32, dins["ryT"][:])

            conv_row = tp.tile([1, GH * GW], F32, tag="convrow")
            for s in range(10):
                cp = ps.tile([128, 2, 512], F32, tag="pst")
                cpv = cp.rearrange("p a b -> p (a b)")[0:1, 0:480]
                for kt in range(2):
                    nc.tensor.matmul(cpv, conv_wT[:, kt:kt + 1], h4[:, kt, s * 480:(s + 1) * 480],
                                     start=(kt == 0), stop=(kt == 1))
                nc.scalar.activation(conv_row[0:1, s * 480:(s + 1) * 480], cpv,
                                     AF.Identity, bias=conv_b[:])
            # G^T [80, 60]: conv tokens are x-major (h4 is [d, x, y]) -> reshape
            gt = tp.tile([GW, GH], F32, tag="gt")
            nc.gpsimd.dma_start(gt[:], conv_row.rearrange("a (x y) -> (a x) y", x=GW))

            # resize rows: T1 [60, 640] = G @ rx
            t1p = ps.tile([128, 2, 512], F32, tag="pst")
            t1v = t1p.rearrange("p a b -> p (a b)")
            nc.tensor.matmul(t1v[0:GH, 0:512], gt[:], rx_t[:, 0:512], start=True, stop=True)
            nc.tensor.matmul(t1v[0:GH, 512:512 + 128], gt[:], rx_t[:, 512:IN_W], start=True, stop=True)
            t1 = tp.tile([GH, IN_W], F32, tag="t1")
            nc.scalar.activation(t1[:], t1v[0:GH, 0:IN_W], AF.Identity)

            # resize cols: out chunk j [120, 640] = ryT[:, j].T @ t1
            # softmax without max-subtraction (logits are O(1); fp32 exp is safe)
            oj = tp.tile([120, 4, IN_W], F32, tag="oj")
            sums = tp.tile([120, 4], F32, tag="sums")
            for j in range(4):
                op_ = ps.tile([128, 2, 512], F32, tag="pst")
                ov = op_.rearrange("p a b -> p (a b)")
                nc.tensor.matmul(ov[0:120, 0:512], ryT_t[:, j * 120:(j + 1) * 120],
                                 t1[:, 0:512], start=True, stop=True)
                nc.tensor.matmul(ov[0:120, 512:IN_W], ryT_t[:, j * 120:(j + 1) * 120],
                                 t1[:, 512:IN_W], start=True, stop=True)
                nc.scalar.activation(oj[:, j, :], ov[0:120, 0:IN_W], AF.Exp)
                nc.vector.tensor_reduce(sums[:, j:j + 1], oj[:, j, :],
                                        mybir.AxisListType.X, ALU.add)
            scol = tp.tile([120, 1], F32, tag="scol")
            nc.vector.tensor_reduce(scol[:], sums[:], mybir.AxisListType.X, ALU.add)
            onescol = tp.tile([120, 1], F32, tag="onescol")
            nc.vector.memset(onescol[:], 1.0)
            onesrow = tp.tile([1, 120], F32, tag="onesrow")
            nc.vector.memset(onesrow[:], 1.0)
            sps = ps.tile([128, 2, 512], F32, tag="pst")
            nc.tensor.matmul(sps[0:1, 0, 0:1], onescol[:], scol[:], start=True, stop=True)
            stot = tp.tile([1, 1], F32, tag="stot")
            nc.vector.reciprocal(stot[:], sps[0:1, 0, 0:1])
            # broadcast 1/S to 120 partitions via K=1 matmul: ones[1,120].T @ stot[1,1]
            rps = ps.tile([128, 2, 512], F32, tag="pst")
            nc.tensor.matmul(rps[0:120, 0, 0:1], onesrow[:], stot[:], start=True, stop=True)
            rb = tp.tile([120, 1], F32, tag="rb")
            nc.vector.tensor_copy(rb[:], rps[0:120, 0, 0:1])
            for j in range(4):
                nc.vector.tensor_scalar(oj[:, j, :], oj[:, j, :], rb[:], None, ALU.mult)
                nc.sync.dma_start(out_dram[j * 120:(j + 1) * 120, :], oj[:, j, :])

    nc.compile()
    return nc


def kernel(**inputs):
    inputs = {k: np.asarray(v) for k, v in inputs.items()}
    debug = bool(int(os.environ.get("BASSK_DEBUG", "0")))
    trace = bool(int(os.environ.get("BASSK_TRACE", "0")))
    key = ("prog", debug)
    if key not in _cache:
        _cache[key] = build_program(debug=debug)
    nc = _cache[key]

    sh = _prep_shared(inputs)
    lf = inputs["lf"]    # [8, 512, 60, 80]
    ctx = inputs["ctx"]  # [8, 128]
    in_maps = []
    for n in range(NCORES):
        m = dict(sh)
        m["lf"] = np.ascontiguousarray(lf[n].transpose(0, 2, 1)).astype(np.float32)
        m["ctx"] = np.ascontiguousarray(ctx[n].reshape(128, 1)).astype(np.float32)
        in_maps.append(m)

    if trace:
        try:
            import antenv
            p = '/opt/trn_rl_repo/antenv'
            if p not in getattr(antenv, "__path__", []):
                antenv.__path__.append(p)
            from trn_agent_boot.trn_boot import _ntff_profile_via_ctypes
            from antenv.axon_hooks import set_axon_ntff_profile_hook
            set_axon_ntff_profile_hook(_ntff_profile_via_ctypes('/opt/axon/libaxon_pjrt.so'))
        except Exception as e:
            print("trace hook setup failed:", e)

    res = run_bass_kernel_spmd(nc, in_maps, core_ids=list(range(NCORES)), trace=trace)
    _cache["last_exec_time_ns"] = res.exec_time_ns
    _cache["last_results"] = res.results
    out = np.stack([res.results[i]["out"][None, :, :] for i in range(NCORES)])
    return out.astype(np.float32)
